# revision 11
# baseline (speedup 1.0000x reference)
"""Pairwise KL divergence kernel for Trainium2, SPMD across 8 NeuronCores.

out[n, m] = sum_d a[n,d]*(log a[n,d] - log b[m,d])
          = ent[n] - (a @ log(b)^T)[n, m],  ent = rowsum(a * log a)

Sharding: a (and output rows) split 8 ways; b replicated.
Per core: a_shard (1024, 64), b (8192, 64) -> out_shard (1024, 8192).

The kernel is output-write bound: 32 MiB of fp32 out per core vs ~358 GB/s
HBM-per-NC => ~94 us floor. Two variants:

v1 (baseline): full b prologue (load + Ln + 64 PE transposes -> lbT), then
  per n-tile GEMM + fused evac + one 4 MiB out DMA. First out DMA waits on
  the whole b prologue.

v2 (pipelined): b processed in 4 chunks of 2048 rows. Per chunk: prefetch
  next chunk's DMA (ACT ring, separate HWDGE FIFO from the out stores),
  Ln + 16 PE transposes, then for each of 8 n-tiles 4 matmuls (512 cols)
  + fused evac (-psum + ent, alternating ACT/DVE), out DMA per OUT_GROUP
  n-tiles (OUT_GROUP MiB each). Out DMAs start ~8 us in instead of ~25+.
"""

import numpy as np

N, M, D = 8192, 8192, 64
NCORES = 8
NSHARD = N // NCORES          # 1024 rows of a per core
NT = NSHARD // 128            # 8 n-tiles per core
MT = M // 512                 # 16 m-tiles of 512
BT = M // 128                 # 64 b row-tiles to transpose
NCHUNK = 4                    # v2: b chunks
BT_H = BT // NCHUNK           # 16 b tiles per chunk (2048 rows)

# matmul operand dtype: "fp32" (exact, 4 cyc/row) or "fp32r" (1 cyc/row)
MM_DTYPE = "fp32r"
VARIANT = "v2"
OUT_GROUP = 2                 # v2: n-tiles per out DMA (1, 2, 4, or 8)
REPEAT = 1                    # bench only: body repetitions inside one NEFF

_CACHE = {}


def _build_v1(nc, tile, mybir, make_identity, mm_dtype, a_d, b_d, out_d):
    from contextlib import ExitStack

    FP32 = mybir.dt.float32
    AF = mybir.ActivationFunctionType
    ALU = mybir.AluOpType
    AX = mybir.AxisListType

    # fp32r matmul operands must be *produced* as float32r (the BIR verifier
    # requires the producing instruction to round) — so the aT/lbT staging
    # tiles themselves carry the matmul dtype and the PSUM->SBUF copies cast.
    MMDT = mybir.dt.float32r if mm_dtype == "fp32r" else FP32
    B_CHUNK = 16

    with tile.TileContext(nc) as tc, ExitStack() as ctx:
        consts = ctx.enter_context(tc.tile_pool(name="consts", bufs=1))
        apool = ctx.enter_context(tc.tile_pool(name="apool", bufs=1))
        bpool = ctx.enter_context(tc.tile_pool(name="bpool", bufs=2))
        lbtp = ctx.enter_context(tc.tile_pool(name="lbtp", bufs=1))
        tpsum = ctx.enter_context(tc.tile_pool(name="tpsum", bufs=2, space="PSUM"))
        mmps = ctx.enter_context(tc.tile_pool(name="mmps", bufs=3, space="PSUM"))
        stage = ctx.enter_context(tc.tile_pool(name="stage", bufs=2))

        ident = consts.tile([128, 128], FP32)
        make_identity(nc, ident)
        # Dummy transpose so PE observes the gpsimd (ident) sem here: the
        # matmul/LDW struct only carries ONE sync wait, so later transposes
        # must each need at most one sem (codegen: "Too many sync waits").
        warm = tpsum.tile([128, 128], FP32, tag="tp")
        nc.tensor.transpose(warm, ident, ident)

        # ---------------- a prologue ----------------
        a_nat = apool.tile([128, NT, D], FP32)        # row t*128+p at [p, t, :]
        nc.sync.dma_start(out=a_nat, in_=a_d[:, :].rearrange("(t p) d -> p t d", p=128))
        la = apool.tile([128, NT, D], FP32)
        nc.scalar.activation(la, a_nat, AF.Ln)
        prod = apool.tile([128, NT, D], FP32)
        nc.vector.tensor_mul(prod, a_nat, la)
        ent = apool.tile([128, NT], FP32)
        for t in range(NT):
            nc.vector.reduce_sum(ent[:, t : t + 1], prod[:, t, :], axis=AX.X)
        aT = apool.tile([64, NT, 128], MMDT)          # aT[:, t, :] = a tile t transposed
        for g in range(2):
            tp = tpsum.tile([64, 4, 128], FP32, tag="tp")
            for j in range(4):
                nc.tensor.transpose(tp[:, j], a_nat[:, g * 4 + j, :], ident)
            nc.scalar.copy(aT[:, g * 4 : (g + 1) * 4, :], tp)

        # ---------------- b prologue ----------------
        lbT = lbtp.tile([64, BT, 128], MMDT)          # lbT[:, bt, :] = lb tile bt transposed
        b_r = b_d[:, :].rearrange("(t p) d -> p t d", p=128)
        n_chunks = BT // B_CHUNK
        for h in range(n_chunks):
            b_nat = bpool.tile([128, B_CHUNK, D], FP32, tag="b_nat")
            nc.sync.dma_start(out=b_nat, in_=b_r[:, h * B_CHUNK : (h + 1) * B_CHUNK, :])
            lb = bpool.tile([128, B_CHUNK, D], FP32, tag="lb")
            nc.scalar.activation(lb, b_nat, AF.Ln)
            for gg in range(B_CHUNK // 4):
                bt0 = h * B_CHUNK + gg * 4
                tp = tpsum.tile([64, 4, 128], FP32, tag="tp")
                for j in range(4):
                    nc.tensor.transpose(tp[:, j], lb[:, gg * 4 + j, :], ident)
                nc.scalar.copy(lbT[:, bt0 : bt0 + 4, :], tp)

        # ---------------- main GEMM + fused evac ----------------
        for t in range(NT):
            out_sb = stage.tile([128, MT, 512], FP32, tag="out_sb")
            lhsT = aT[:, t, :]
            ent_t = ent[:, t : t + 1]
            for g in range(MT // 2):
                ps = mmps.tile([128, 2, 512], FP32, tag="ps")
                for j in range(2):
                    mi = g * 2 + j
                    nc.tensor.matmul(
                        ps[:, j],
                        lhsT,
                        lbT[:, mi * 4 : (mi + 1) * 4, :],
                        start=True,
                        stop=True,
                    )
                dst = out_sb[:, g * 2 : (g + 1) * 2, :]
                if g % 2 == 0:
                    nc.scalar.activation(dst, ps, AF.Identity, bias=ent_t, scale=-1.0)
                else:
                    nc.vector.tensor_scalar(dst, ps, -1.0, ent_t, ALU.mult, ALU.add)
            nc.sync.dma_start(
                out=out_d[t * 128 : (t + 1) * 128, :].rearrange(
                    "p (c m) -> p c m", m=512
                ),
                in_=out_sb,
            )


def _build_v2(nc, tile, mybir, make_identity, mm_dtype, out_group, a_d, b_d, out_d):
    from contextlib import ExitStack

    FP32 = mybir.dt.float32
    AF = mybir.ActivationFunctionType
    ALU = mybir.AluOpType
    AX = mybir.AxisListType

    MMDT = mybir.dt.float32r if mm_dtype == "fp32r" else FP32
    MC = BT_H * 128               # 2048 columns per chunk
    GC = MC // 512                # 4 psum banks of 512 per chunk-row
    NG = NT // out_group          # out DMAs per chunk

    with tile.TileContext(nc) as tc, ExitStack() as ctx:
        consts = ctx.enter_context(tc.tile_pool(name="consts", bufs=1))
        apool = ctx.enter_context(tc.tile_pool(name="apool", bufs=1))
        bpool = ctx.enter_context(tc.tile_pool(name="bpool", bufs=2))
        lbtp = ctx.enter_context(tc.tile_pool(name="lbtp", bufs=2))
        tpsum = ctx.enter_context(tc.tile_pool(name="tpsum", bufs=2, space="PSUM"))
        mmps = ctx.enter_context(tc.tile_pool(name="mmps", bufs=3, space="PSUM"))
        stage = ctx.enter_context(tc.tile_pool(name="stage", bufs=2))

        ident = consts.tile([128, 128], FP32)
        make_identity(nc, ident)
        # Dummy transpose so PE observes the gpsimd (ident) sem here (the
        # matmul/LDW struct only carries ONE sync wait).
        warm = tpsum.tile([128, 128], FP32, tag="tp")
        nc.tensor.transpose(warm, ident, ident)

        b_r = b_d[:, :].rearrange("(t p) d -> p t d", p=128)

        # ---------------- a prologue + b chunk-0 prefetch ----------------
        b_nats = [None] * NCHUNK
        b_nats[0] = bpool.tile([128, BT_H, D], FP32, tag="b_nat", name="b_nat")
        nc.scalar.dma_start(out=b_nats[0], in_=b_r[:, :BT_H, :])

        a_nat = apool.tile([128, NT, D], FP32)        # row t*128+p at [p, t, :]
        nc.sync.dma_start(out=a_nat, in_=a_d[:, :].rearrange("(t p) d -> p t d", p=128))
        la = apool.tile([128, NT, D], FP32)
        nc.scalar.activation(la, a_nat, AF.Ln)
        prod = apool.tile([128, NT, D], FP32)
        nc.vector.tensor_mul(prod, a_nat, la)
        ent = apool.tile([128, NT], FP32)
        for t in range(NT):
            nc.vector.reduce_sum(ent[:, t : t + 1], prod[:, t, :], axis=AX.X)
        aT = apool.tile([64, NT, 128], MMDT)          # aT[:, t, :] = a tile t transposed
        for g in range(2):
            tp = tpsum.tile([64, 4, 128], FP32, tag="tp")
            for j in range(4):
                nc.tensor.transpose(tp[:, j], a_nat[:, g * 4 + j, :], ident)
            nc.scalar.copy(aT[:, g * 4 : (g + 1) * 4, :], tp)

        # ---------------- chunk-pipelined GEMM ----------------
        for h in range(NCHUNK):
            if h + 1 < NCHUNK:
                b_nats[h + 1] = bpool.tile(
                    [128, BT_H, D], FP32, tag="b_nat", name="b_nat"
                )
                nc.scalar.dma_start(
                    out=b_nats[h + 1],
                    in_=b_r[:, (h + 1) * BT_H : (h + 2) * BT_H, :],
                )
            lb = bpool.tile([128, BT_H, D], FP32, tag="lb")
            nc.scalar.activation(lb, b_nats[h], AF.Ln)
            lbT = lbtp.tile([64, BT_H, 128], MMDT, tag="lbT")
            for gg in range(BT_H // 4):
                tp = tpsum.tile([64, 4, 128], FP32, tag="tp")
                for j in range(4):
                    nc.tensor.transpose(tp[:, j], lb[:, gg * 4 + j, :], ident)
                nc.scalar.copy(lbT[:, gg * 4 : (gg + 1) * 4, :], tp)

            for tg in range(NG):
                out_sb = stage.tile([128, out_group, GC, 512], FP32, tag="out_sb")
                for u in range(out_group):
                    t = tg * out_group + u
                    lhsT = aT[:, t, :]
                    ent_t = ent[:, t : t + 1]
                    for g in range(GC // 2):
                        ps = mmps.tile([128, 2, 512], FP32, tag="ps")
                        for j in range(2):
                            mi = g * 2 + j
                            nc.tensor.matmul(
                                ps[:, j],
                                lhsT,
                                lbT[:, mi * 4 : (mi + 1) * 4, :],
                                start=True,
                                stop=True,
                            )
                        dst = out_sb[:, u, g * 2 : (g + 1) * 2, :]
                        if (t * (GC // 2) + g) % 2 == 0:
                            nc.scalar.activation(
                                dst, ps, AF.Identity, bias=ent_t, scale=-1.0
                            )
                        else:
                            nc.vector.tensor_scalar(
                                dst, ps, -1.0, ent_t, ALU.mult, ALU.add
                            )
                t0 = tg * out_group
                nc.sync.dma_start(
                    out=out_d[
                        t0 * 128 : (t0 + out_group) * 128,
                        h * MC : (h + 1) * MC,
                    ].rearrange("(u p) (c m) -> p u c m", p=128, m=512),
                    in_=out_sb,
                )


def _build_v3(nc, tile, mybir, make_identity, mm_dtype, out_group, a_d, b_d, out_d):
    """v2 + variable b-chunk schedule: small leading chunks start the out
    DMAs ~5 us earlier; large trailing chunks keep 1-2 MiB DMA efficiency."""
    from contextlib import ExitStack

    FP32 = mybir.dt.float32
    AF = mybir.ActivationFunctionType
    ALU = mybir.AluOpType
    AX = mybir.AxisListType

    MMDT = mybir.dt.float32r if mm_dtype == "fp32r" else FP32
    CHUNKS = [4, 4, 8, 16, 16, 16]            # b tiles per chunk, sums to BT
    assert sum(CHUNKS) == BT
    starts = [sum(CHUNKS[:i]) for i in range(len(CHUNKS))]
    NG = NT // out_group

    with tile.TileContext(nc) as tc, ExitStack() as ctx:
        consts = ctx.enter_context(tc.tile_pool(name="consts", bufs=1))
        apool = ctx.enter_context(tc.tile_pool(name="apool", bufs=1))
        bpool = ctx.enter_context(tc.tile_pool(name="bpool", bufs=2))
        lbtp = ctx.enter_context(tc.tile_pool(name="lbtp", bufs=2))
        tpsum = ctx.enter_context(tc.tile_pool(name="tpsum", bufs=2, space="PSUM"))
        mmps = ctx.enter_context(tc.tile_pool(name="mmps", bufs=2, space="PSUM"))
        stage = ctx.enter_context(tc.tile_pool(name="stage", bufs=2))

        ident = consts.tile([128, 128], FP32)
        make_identity(nc, ident)
        warm = tpsum.tile([128, 128], FP32, tag="tp")
        nc.tensor.transpose(warm, ident, ident)

        b_r = b_d[:, :].rearrange("(t p) d -> p t d", p=128)

        # ---------------- a prologue + b chunk-0 prefetch ----------------
        b_nats = [None] * len(CHUNKS)
        b_nats[0] = bpool.tile([128, CHUNKS[0], D], FP32, tag="b_nat", name="b_nat")
        nc.scalar.dma_start(out=b_nats[0], in_=b_r[:, : CHUNKS[0], :])

        a_nat = apool.tile([128, NT, D], FP32)
        nc.sync.dma_start(out=a_nat, in_=a_d[:, :].rearrange("(t p) d -> p t d", p=128))
        la = apool.tile([128, NT, D], FP32)
        nc.scalar.activation(la, a_nat, AF.Ln)
        prod = apool.tile([128, NT, D], FP32)
        nc.vector.tensor_mul(prod, a_nat, la)
        ent = apool.tile([128, NT], FP32)
        for t in range(NT):
            nc.vector.reduce_sum(ent[:, t : t + 1], prod[:, t, :], axis=AX.X)
        aT = apool.tile([64, NT, 128], MMDT)
        for g in range(2):
            tp = tpsum.tile([64, 4, 128], FP32, tag="tp")
            for j in range(4):
                nc.tensor.transpose(tp[:, j], a_nat[:, g * 4 + j, :], ident)
            nc.scalar.copy(aT[:, g * 4 : (g + 1) * 4, :], tp)

        # ---------------- chunk-pipelined GEMM ----------------
        evac_flip = 0
        for h, nbt in enumerate(CHUNKS):
            if h + 1 < len(CHUNKS):
                b_nats[h + 1] = bpool.tile(
                    [128, CHUNKS[h + 1], D], FP32, tag="b_nat", name="b_nat"
                )
                nc.scalar.dma_start(
                    out=b_nats[h + 1],
                    in_=b_r[:, starts[h + 1] : starts[h + 1] + CHUNKS[h + 1], :],
                )
            lb = bpool.tile([128, nbt, D], FP32, tag="lb")
            nc.scalar.activation(lb, b_nats[h], AF.Ln)
            lbT = lbtp.tile([64, nbt, 128], MMDT, tag="lbT")
            for gg in range(nbt // 4):
                tp = tpsum.tile([64, 4, 128], FP32, tag="tp")
                for j in range(4):
                    nc.tensor.transpose(tp[:, j], lb[:, gg * 4 + j, :], ident)
                nc.scalar.copy(lbT[:, gg * 4 : (gg + 1) * 4, :], tp)

            nbank = nbt // 4                  # 512-col psum banks per t
            for tg in range(NG):
                out_sb = stage.tile([128, out_group, nbank, 512], FP32, tag="out_sb")
                for u in range(out_group):
                    t = tg * out_group + u
                    lhsT = aT[:, t, :]
                    ent_t = ent[:, t : t + 1]
                    g = 0
                    while g < nbank:
                        nb = min(2, nbank - g)
                        ps = mmps.tile([128, nb, 512], FP32, tag=f"ps{nb}")
                        for j in range(nb):
                            mi = g + j
                            nc.tensor.matmul(
                                ps[:, j],
                                lhsT,
                                lbT[:, mi * 4 : (mi + 1) * 4, :],
                                start=True,
                                stop=True,
                            )
                        dst = out_sb[:, u, g : g + nb, :]
                        if evac_flip % 2 == 0:
                            nc.scalar.activation(
                                dst, ps, AF.Identity, bias=ent_t, scale=-1.0
                            )
                        else:
                            nc.vector.tensor_scalar(
                                dst, ps, -1.0, ent_t, ALU.mult, ALU.add
                            )
                        evac_flip += 1
                        g += nb
                t0 = tg * out_group
                nc.sync.dma_start(
                    out=out_d[
                        t0 * 128 : (t0 + out_group) * 128,
                        starts[h] * 128 : (starts[h] + nbt) * 128,
                    ].rearrange("(u p) (c m) -> p u c m", p=128, m=512),
                    in_=out_sb,
                )


def _build(mm_dtype, variant, out_group, repeat=1):
    import concourse.bacc as bacc_mod
    import concourse.mybir as mybir
    import concourse.tile as tile
    from concourse.masks import make_identity

    nc = bacc_mod.Bacc()
    FP32 = mybir.dt.float32
    a_d = nc.dram_tensor("a", [NSHARD, D], FP32, kind="ExternalInput")
    b_d = nc.dram_tensor("b", [M, D], FP32, kind="ExternalInput")
    out_d = nc.dram_tensor("out", [NSHARD, M], FP32, kind="ExternalOutput")
    for r in range(repeat):
        if variant == "v1":
            _build_v1(nc, tile, mybir, make_identity, mm_dtype, a_d, b_d, out_d)
        elif variant == "v2":
            _build_v2(
                nc, tile, mybir, make_identity, mm_dtype, out_group, a_d, b_d, out_d
            )
        else:
            _build_v3(
                nc, tile, mybir, make_identity, mm_dtype, out_group, a_d, b_d, out_d
            )
    # bacc lowering: splits multi-sem waits onto event-semaphore/nop
    # instructions (HW allows one sync wait per engine instruction).
    nc.compile()
    return nc


def _get_nc():
    key = (MM_DTYPE, VARIANT, OUT_GROUP, REPEAT)
    if key not in _CACHE:
        _CACHE[key] = _build(*key)
    return _CACHE[key]


def _run(a, b, trace=False):
    from concourse.bass_utils import run_bass_kernel_spmd

    nc = _get_nc()
    a = np.ascontiguousarray(np.asarray(a, dtype=np.float32))
    b = np.ascontiguousarray(np.asarray(b, dtype=np.float32))
    in_maps = [
        {"a": a[i * NSHARD : (i + 1) * NSHARD], "b": b} for i in range(NCORES)
    ]
    res = run_bass_kernel_spmd(nc, in_maps, list(range(NCORES)), trace=trace)
    out = np.concatenate([r["out"] for r in res.results], axis=0)
    return out, res


def kernel(a, b):
    out, _ = _run(a, b, trace=False)
    return out


# revision 14
# speedup vs baseline: 1.1264x; 1.1264x over previous
"""Pairwise KL divergence kernel for Trainium2, SPMD across 8 NeuronCores.

out[n, m] = sum_d a[n,d]*(log a[n,d] - log b[m,d])
          = ent[n] - (a @ log(b)^T)[n, m],  ent = rowsum(a * log a)

Sharding: a (and output rows) split 8 ways; b replicated.
Per core: a_shard (1024, 64), b (8192, 64) -> out_shard (1024, 8192).

The kernel is output-write bound: 32 MiB of fp32 out per core vs ~358 GB/s
HBM-per-NC => ~94 us floor. Two variants:

v1 (baseline): full b prologue (load + Ln + 64 PE transposes -> lbT), then
  per n-tile GEMM + fused evac + one 4 MiB out DMA. First out DMA waits on
  the whole b prologue.

v2 (pipelined): b processed in 4 chunks of 2048 rows. Per chunk: prefetch
  next chunk's DMA (ACT ring, separate HWDGE FIFO from the out stores),
  Ln + 16 PE transposes, then for each of 8 n-tiles 4 matmuls (512 cols)
  + fused evac (-psum + ent, alternating ACT/DVE), out DMA per OUT_GROUP
  n-tiles (OUT_GROUP MiB each). Out DMAs start ~8 us in instead of ~25+.
"""

import numpy as np

N, M, D = 8192, 8192, 64
NCORES = 8
NSHARD = N // NCORES          # 1024 rows of a per core
NT = NSHARD // 128            # 8 n-tiles per core
MT = M // 512                 # 16 m-tiles of 512
BT = M // 128                 # 64 b row-tiles to transpose
NCHUNK = 4                    # v2: b chunks
BT_H = BT // NCHUNK           # 16 b tiles per chunk (2048 rows)

# matmul operand dtype: "fp32" (exact, 4 cyc/row) or "fp32r" (1 cyc/row)
MM_DTYPE = "fp32r"
VARIANT = "v2"
OUT_GROUP = 2                 # v2: n-tiles per out DMA (1, 2, 4, or 8)
OUT_RING = "sync"             # "sync" = all out DMAs on SP ring; "alt" = alternate SP/ACT
REPEAT = 1                    # bench only: body repetitions inside one NEFF

_CACHE = {}


def _build_v1(nc, tile, mybir, make_identity, mm_dtype, a_d, b_d, out_d):
    from contextlib import ExitStack

    FP32 = mybir.dt.float32
    AF = mybir.ActivationFunctionType
    ALU = mybir.AluOpType
    AX = mybir.AxisListType

    # fp32r matmul operands must be *produced* as float32r (the BIR verifier
    # requires the producing instruction to round) — so the aT/lbT staging
    # tiles themselves carry the matmul dtype and the PSUM->SBUF copies cast.
    MMDT = mybir.dt.float32r if mm_dtype == "fp32r" else FP32
    B_CHUNK = 16

    with tile.TileContext(nc) as tc, ExitStack() as ctx:
        consts = ctx.enter_context(tc.tile_pool(name="consts", bufs=1))
        apool = ctx.enter_context(tc.tile_pool(name="apool", bufs=1))
        bpool = ctx.enter_context(tc.tile_pool(name="bpool", bufs=2))
        lbtp = ctx.enter_context(tc.tile_pool(name="lbtp", bufs=1))
        tpsum = ctx.enter_context(tc.tile_pool(name="tpsum", bufs=2, space="PSUM"))
        mmps = ctx.enter_context(tc.tile_pool(name="mmps", bufs=3, space="PSUM"))
        stage = ctx.enter_context(tc.tile_pool(name="stage", bufs=2))

        ident = consts.tile([128, 128], FP32)
        make_identity(nc, ident)
        # Dummy transpose so PE observes the gpsimd (ident) sem here: the
        # matmul/LDW struct only carries ONE sync wait, so later transposes
        # must each need at most one sem (codegen: "Too many sync waits").
        warm = tpsum.tile([128, 128], FP32, tag="tp")
        nc.tensor.transpose(warm, ident, ident)

        # ---------------- a prologue ----------------
        a_nat = apool.tile([128, NT, D], FP32)        # row t*128+p at [p, t, :]
        nc.sync.dma_start(out=a_nat, in_=a_d[:, :].rearrange("(t p) d -> p t d", p=128))
        la = apool.tile([128, NT, D], FP32)
        nc.scalar.activation(la, a_nat, AF.Ln)
        prod = apool.tile([128, NT, D], FP32)
        nc.vector.tensor_mul(prod, a_nat, la)
        ent = apool.tile([128, NT], FP32)
        for t in range(NT):
            nc.vector.reduce_sum(ent[:, t : t + 1], prod[:, t, :], axis=AX.X)
        aT = apool.tile([64, NT, 128], MMDT)          # aT[:, t, :] = a tile t transposed
        for g in range(2):
            tp = tpsum.tile([64, 4, 128], FP32, tag="tp")
            for j in range(4):
                nc.tensor.transpose(tp[:, j], a_nat[:, g * 4 + j, :], ident)
            nc.scalar.copy(aT[:, g * 4 : (g + 1) * 4, :], tp)

        # ---------------- b prologue ----------------
        lbT = lbtp.tile([64, BT, 128], MMDT)          # lbT[:, bt, :] = lb tile bt transposed
        b_r = b_d[:, :].rearrange("(t p) d -> p t d", p=128)
        n_chunks = BT // B_CHUNK
        for h in range(n_chunks):
            b_nat = bpool.tile([128, B_CHUNK, D], FP32, tag="b_nat")
            nc.sync.dma_start(out=b_nat, in_=b_r[:, h * B_CHUNK : (h + 1) * B_CHUNK, :])
            lb = bpool.tile([128, B_CHUNK, D], FP32, tag="lb")
            nc.scalar.activation(lb, b_nat, AF.Ln)
            for gg in range(B_CHUNK // 4):
                bt0 = h * B_CHUNK + gg * 4
                tp = tpsum.tile([64, 4, 128], FP32, tag="tp")
                for j in range(4):
                    nc.tensor.transpose(tp[:, j], lb[:, gg * 4 + j, :], ident)
                nc.scalar.copy(lbT[:, bt0 : bt0 + 4, :], tp)

        # ---------------- main GEMM + fused evac ----------------
        for t in range(NT):
            out_sb = stage.tile([128, MT, 512], FP32, tag="out_sb")
            lhsT = aT[:, t, :]
            ent_t = ent[:, t : t + 1]
            for g in range(MT // 2):
                ps = mmps.tile([128, 2, 512], FP32, tag="ps")
                for j in range(2):
                    mi = g * 2 + j
                    nc.tensor.matmul(
                        ps[:, j],
                        lhsT,
                        lbT[:, mi * 4 : (mi + 1) * 4, :],
                        start=True,
                        stop=True,
                    )
                dst = out_sb[:, g * 2 : (g + 1) * 2, :]
                if g % 2 == 0:
                    nc.scalar.activation(dst, ps, AF.Identity, bias=ent_t, scale=-1.0)
                else:
                    nc.vector.tensor_scalar(dst, ps, -1.0, ent_t, ALU.mult, ALU.add)
            nc.sync.dma_start(
                out=out_d[t * 128 : (t + 1) * 128, :].rearrange(
                    "p (c m) -> p c m", m=512
                ),
                in_=out_sb,
            )


def _build_v2(nc, tile, mybir, make_identity, mm_dtype, out_group, a_d, b_d, out_d):
    from contextlib import ExitStack

    FP32 = mybir.dt.float32
    AF = mybir.ActivationFunctionType
    ALU = mybir.AluOpType
    AX = mybir.AxisListType

    MMDT = mybir.dt.float32r if mm_dtype == "fp32r" else FP32
    MC = BT_H * 128               # 2048 columns per chunk
    GC = MC // 512                # 4 psum banks of 512 per chunk-row
    NG = NT // out_group          # out DMAs per chunk

    with tile.TileContext(nc) as tc, ExitStack() as ctx:
        consts = ctx.enter_context(tc.tile_pool(name="consts", bufs=1))
        apool = ctx.enter_context(tc.tile_pool(name="apool", bufs=1))
        bpool = ctx.enter_context(tc.tile_pool(name="bpool", bufs=2))
        lbtp = ctx.enter_context(tc.tile_pool(name="lbtp", bufs=2))
        tpsum = ctx.enter_context(tc.tile_pool(name="tpsum", bufs=2, space="PSUM"))
        mmps = ctx.enter_context(tc.tile_pool(name="mmps", bufs=3, space="PSUM"))
        stage = ctx.enter_context(tc.tile_pool(name="stage", bufs=2))

        ident = consts.tile([128, 128], FP32)
        make_identity(nc, ident)
        # Dummy transpose so PE observes the gpsimd (ident) sem here (the
        # matmul/LDW struct only carries ONE sync wait).
        warm = tpsum.tile([128, 128], FP32, tag="tp")
        nc.tensor.transpose(warm, ident, ident)

        b_r = b_d[:, :].rearrange("(t p) d -> p t d", p=128)

        # ---------------- a prologue + b chunk-0 prefetch ----------------
        b_nats = [None] * NCHUNK
        b_nats[0] = bpool.tile([128, BT_H, D], FP32, tag="b_nat", name="b_nat")
        nc.scalar.dma_start(out=b_nats[0], in_=b_r[:, :BT_H, :])

        a_nat = apool.tile([128, NT, D], FP32)        # row t*128+p at [p, t, :]
        nc.sync.dma_start(out=a_nat, in_=a_d[:, :].rearrange("(t p) d -> p t d", p=128))
        la = apool.tile([128, NT, D], FP32)
        nc.scalar.activation(la, a_nat, AF.Ln)
        prod = apool.tile([128, NT, D], FP32)
        nc.vector.tensor_mul(prod, a_nat, la)
        ent = apool.tile([128, NT], FP32)
        for t in range(NT):
            nc.vector.reduce_sum(ent[:, t : t + 1], prod[:, t, :], axis=AX.X)
        aT = apool.tile([64, NT, 128], MMDT)          # aT[:, t, :] = a tile t transposed
        for g in range(2):
            tp = tpsum.tile([64, 4, 128], FP32, tag="tp")
            for j in range(4):
                nc.tensor.transpose(tp[:, j], a_nat[:, g * 4 + j, :], ident)
            nc.scalar.copy(aT[:, g * 4 : (g + 1) * 4, :], tp)

        # ---------------- chunk-pipelined GEMM ----------------
        for h in range(NCHUNK):
            if h + 1 < NCHUNK:
                b_nats[h + 1] = bpool.tile(
                    [128, BT_H, D], FP32, tag="b_nat", name="b_nat"
                )
                nc.scalar.dma_start(
                    out=b_nats[h + 1],
                    in_=b_r[:, (h + 1) * BT_H : (h + 2) * BT_H, :],
                )
            lb = bpool.tile([128, BT_H, D], FP32, tag="lb")
            nc.scalar.activation(lb, b_nats[h], AF.Ln)
            lbT = lbtp.tile([64, BT_H, 128], MMDT, tag="lbT")
            for gg in range(BT_H // 4):
                tp = tpsum.tile([64, 4, 128], FP32, tag="tp")
                for j in range(4):
                    nc.tensor.transpose(tp[:, j], lb[:, gg * 4 + j, :], ident)
                nc.scalar.copy(lbT[:, gg * 4 : (gg + 1) * 4, :], tp)

            for tg in range(NG):
                out_sb = stage.tile([128, out_group, GC, 512], FP32, tag="out_sb")
                for u in range(out_group):
                    t = tg * out_group + u
                    lhsT = aT[:, t, :]
                    ent_t = ent[:, t : t + 1]
                    for g in range(GC // 2):
                        ps = mmps.tile([128, 2, 512], FP32, tag="ps")
                        for j in range(2):
                            mi = g * 2 + j
                            nc.tensor.matmul(
                                ps[:, j],
                                lhsT,
                                lbT[:, mi * 4 : (mi + 1) * 4, :],
                                start=True,
                                stop=True,
                            )
                        dst = out_sb[:, u, g * 2 : (g + 1) * 2, :]
                        if (t * (GC // 2) + g) % 2 == 0:
                            nc.scalar.activation(
                                dst, ps, AF.Identity, bias=ent_t, scale=-1.0
                            )
                        else:
                            nc.vector.tensor_scalar(
                                dst, ps, -1.0, ent_t, ALU.mult, ALU.add
                            )
                t0 = tg * out_group
                eng = nc.sync if (OUT_RING == "sync" or tg % 2 == 0) else nc.scalar
                eng.dma_start(
                    out=out_d[
                        t0 * 128 : (t0 + out_group) * 128,
                        h * MC : (h + 1) * MC,
                    ].rearrange("(u p) (c m) -> p u c m", p=128, m=512),
                    in_=out_sb,
                )


def _build_v3(nc, tile, mybir, make_identity, mm_dtype, out_group, a_d, b_d, out_d):
    """v2 + variable b-chunk schedule: small leading chunks start the out
    DMAs ~5 us earlier; large trailing chunks keep 1-2 MiB DMA efficiency."""
    from contextlib import ExitStack

    FP32 = mybir.dt.float32
    AF = mybir.ActivationFunctionType
    ALU = mybir.AluOpType
    AX = mybir.AxisListType

    MMDT = mybir.dt.float32r if mm_dtype == "fp32r" else FP32
    CHUNKS = [4, 4, 8, 16, 16, 16]            # b tiles per chunk, sums to BT
    assert sum(CHUNKS) == BT
    starts = [sum(CHUNKS[:i]) for i in range(len(CHUNKS))]
    NG = NT // out_group

    with tile.TileContext(nc) as tc, ExitStack() as ctx:
        consts = ctx.enter_context(tc.tile_pool(name="consts", bufs=1))
        apool = ctx.enter_context(tc.tile_pool(name="apool", bufs=1))
        bpool = ctx.enter_context(tc.tile_pool(name="bpool", bufs=2))
        lbtp = ctx.enter_context(tc.tile_pool(name="lbtp", bufs=2))
        tpsum = ctx.enter_context(tc.tile_pool(name="tpsum", bufs=2, space="PSUM"))
        mmps = ctx.enter_context(tc.tile_pool(name="mmps", bufs=2, space="PSUM"))
        stage = ctx.enter_context(tc.tile_pool(name="stage", bufs=2))

        ident = consts.tile([128, 128], FP32)
        make_identity(nc, ident)
        warm = tpsum.tile([128, 128], FP32, tag="tp")
        nc.tensor.transpose(warm, ident, ident)

        b_r = b_d[:, :].rearrange("(t p) d -> p t d", p=128)

        # ---------------- a prologue + b chunk-0 prefetch ----------------
        b_nats = [None] * len(CHUNKS)
        b_nats[0] = bpool.tile([128, CHUNKS[0], D], FP32, tag="b_nat", name="b_nat")
        nc.scalar.dma_start(out=b_nats[0], in_=b_r[:, : CHUNKS[0], :])

        a_nat = apool.tile([128, NT, D], FP32)
        nc.sync.dma_start(out=a_nat, in_=a_d[:, :].rearrange("(t p) d -> p t d", p=128))
        la = apool.tile([128, NT, D], FP32)
        nc.scalar.activation(la, a_nat, AF.Ln)
        prod = apool.tile([128, NT, D], FP32)
        nc.vector.tensor_mul(prod, a_nat, la)
        ent = apool.tile([128, NT], FP32)
        for t in range(NT):
            nc.vector.reduce_sum(ent[:, t : t + 1], prod[:, t, :], axis=AX.X)
        aT = apool.tile([64, NT, 128], MMDT)
        for g in range(2):
            tp = tpsum.tile([64, 4, 128], FP32, tag="tp")
            for j in range(4):
                nc.tensor.transpose(tp[:, j], a_nat[:, g * 4 + j, :], ident)
            nc.scalar.copy(aT[:, g * 4 : (g + 1) * 4, :], tp)

        # ---------------- chunk-pipelined GEMM ----------------
        evac_flip = 0
        for h, nbt in enumerate(CHUNKS):
            if h + 1 < len(CHUNKS):
                b_nats[h + 1] = bpool.tile(
                    [128, CHUNKS[h + 1], D], FP32, tag="b_nat", name="b_nat"
                )
                nc.scalar.dma_start(
                    out=b_nats[h + 1],
                    in_=b_r[:, starts[h + 1] : starts[h + 1] + CHUNKS[h + 1], :],
                )
            lb = bpool.tile([128, nbt, D], FP32, tag="lb")
            nc.scalar.activation(lb, b_nats[h], AF.Ln)
            lbT = lbtp.tile([64, nbt, 128], MMDT, tag="lbT")
            for gg in range(nbt // 4):
                tp = tpsum.tile([64, 4, 128], FP32, tag="tp")
                for j in range(4):
                    nc.tensor.transpose(tp[:, j], lb[:, gg * 4 + j, :], ident)
                nc.scalar.copy(lbT[:, gg * 4 : (gg + 1) * 4, :], tp)

            nbank = nbt // 4                  # 512-col psum banks per t
            for tg in range(NG):
                out_sb = stage.tile([128, out_group, nbank, 512], FP32, tag="out_sb")
                for u in range(out_group):
                    t = tg * out_group + u
                    lhsT = aT[:, t, :]
                    ent_t = ent[:, t : t + 1]
                    g = 0
                    while g < nbank:
                        nb = min(2, nbank - g)
                        ps = mmps.tile([128, nb, 512], FP32, tag=f"ps{nb}")
                        for j in range(nb):
                            mi = g + j
                            nc.tensor.matmul(
                                ps[:, j],
                                lhsT,
                                lbT[:, mi * 4 : (mi + 1) * 4, :],
                                start=True,
                                stop=True,
                            )
                        dst = out_sb[:, u, g : g + nb, :]
                        if evac_flip % 2 == 0:
                            nc.scalar.activation(
                                dst, ps, AF.Identity, bias=ent_t, scale=-1.0
                            )
                        else:
                            nc.vector.tensor_scalar(
                                dst, ps, -1.0, ent_t, ALU.mult, ALU.add
                            )
                        evac_flip += 1
                        g += nb
                t0 = tg * out_group
                nc.sync.dma_start(
                    out=out_d[
                        t0 * 128 : (t0 + out_group) * 128,
                        starts[h] * 128 : (starts[h] + nbt) * 128,
                    ].rearrange("(u p) (c m) -> p u c m", p=128, m=512),
                    in_=out_sb,
                )


def _build(mm_dtype, variant, out_group, repeat=1):
    import concourse.bacc as bacc_mod
    import concourse.mybir as mybir
    import concourse.tile as tile
    from concourse.masks import make_identity

    nc = bacc_mod.Bacc()
    FP32 = mybir.dt.float32
    a_d = nc.dram_tensor("a", [NSHARD, D], FP32, kind="ExternalInput")
    b_d = nc.dram_tensor("b", [M, D], FP32, kind="ExternalInput")
    out_d = nc.dram_tensor("out", [NSHARD, M], FP32, kind="ExternalOutput")
    for r in range(repeat):
        if variant == "v1":
            _build_v1(nc, tile, mybir, make_identity, mm_dtype, a_d, b_d, out_d)
        elif variant == "v2":
            _build_v2(
                nc, tile, mybir, make_identity, mm_dtype, out_group, a_d, b_d, out_d
            )
        else:
            _build_v3(
                nc, tile, mybir, make_identity, mm_dtype, out_group, a_d, b_d, out_d
            )
    # bacc lowering: splits multi-sem waits onto event-semaphore/nop
    # instructions (HW allows one sync wait per engine instruction).
    nc.compile()
    return nc


def _get_nc():
    key = (MM_DTYPE, VARIANT, OUT_GROUP, REPEAT, OUT_RING)
    if key not in _CACHE:
        _CACHE[key] = _build(*key[:4])
    return _CACHE[key]


def _run(a, b, trace=False):
    from concourse.bass_utils import run_bass_kernel_spmd

    nc = _get_nc()
    a = np.ascontiguousarray(np.asarray(a, dtype=np.float32))
    b = np.ascontiguousarray(np.asarray(b, dtype=np.float32))
    in_maps = [
        {"a": a[i * NSHARD : (i + 1) * NSHARD], "b": b} for i in range(NCORES)
    ]
    res = run_bass_kernel_spmd(nc, in_maps, list(range(NCORES)), trace=trace)
    out = np.concatenate([r["out"] for r in res.results], axis=0)
    return out, res


def kernel(a, b):
    out, _ = _run(a, b, trace=False)
    return out
